# revision 28
# baseline (speedup 1.0000x reference)
"""Multi-head attention forward, sharded head-parallel across 8 NeuronCores.

Per core c (heads 2c, 2c+1):
  qT/kT/vT = (x @ W{q,k,v}_c.T).T   quad-pass matmuls (ec-outer) for
                                    stationary reuse, ec-major DMA
  v1       = V in [kpos, hd] layout via DMA-engine transpose (off PE)
  scoresT  = kT_chunk.T @ qT        exact-causal: only cols [qoff,512)
  probsT   = exp(scoresT)           ACT on the live columns only
  triangle mask                     gpsimd affine_select on 128-col diag
  av+rowsum: out.T = [v | 1].T @ probsT  (ones column -> softmax denom)
  normalize: DVE reciprocal + gpsimd partition_broadcast + DVE mul
  out_projT partial per (b,t), interleaved with attention, bf16 output
Host: sum the 8 partial [1024, 4096] outputs, transpose, add bias.
"""
import sys

sys.path.insert(0, "/opt/trn_rl_repo")

import ml_dtypes
import numpy as np

BF16 = ml_dtypes.bfloat16

B, S, D = 2, 2048, 1024
H, HD = 16, 64
NCORES = 8
SEC = 128           # output dims per core per section (2 heads * 64)
BS = B * S          # 4096
NT = BS // 512      # 8 seq tiles of 512
EC = D // 128       # 8 embed chunks
QT = S // 512       # 4 q-tiles per (b,h)
KC = S // 128       # 16 k-chunks per (b,h)

_cache = {}

import os as _os
_NOQOFF = bool(int(_os.environ.get("KV_NOQOFF", "0")))
_DEBUG = bool(int(_os.environ.get("KV_DEBUG", "0")))


def _build(mask_mode):
    import concourse.bass as bass
    import concourse.tile as tile
    from concourse import bacc, mybir

    f32 = mybir.dt.float32
    bf16 = mybir.dt.bfloat16
    Exp = mybir.ActivationFunctionType.Exp

    nc = bacc.Bacc("TRN2", target_bir_lowering=False, debug=False,
                   num_devices=NCORES)

    xT = nc.dram_tensor("xT", [D, BS], bf16, kind="ExternalInput")
    # host pre-packs to partition-major [128, EC, 3*SEC] for a
    # contiguous-per-partition (descriptor-efficient) load
    wqkvT = nc.dram_tensor("wqkvT", [128, EC * 3 * SEC], bf16,
                           kind="ExternalInput")
    woT = nc.dram_tensor("woT", [SEC, D], bf16, kind="ExternalInput")
    # [:, 0:128] identity (PE-mode V transposes); [:, 128:128+2*KC]
    # ones (softmax-denominator columns of vv)
    ident = nc.dram_tensor("ident", [128, 128 + 2 * KC], bf16,
                           kind="ExternalInput")
    if mask_mode == "general":
        maskT = nc.dram_tensor("maskT", [S, S], bf16, kind="ExternalInput")
    out_pT = nc.dram_tensor("out_pT", [D, BS], bf16, kind="ExternalOutput")
    if _DEBUG:
        qkvdbg = nc.dram_tensor("qkvdbg", [3, 128, BS], bf16,
                                kind="ExternalOutput")
        v1dbg = nc.dram_tensor("v1dbg", [B, 128, KC * 2 * 128], bf16,
                               kind="ExternalOutput")
        ocatdbg = nc.dram_tensor("ocatdbg", [128, BS], bf16,
                                 kind="ExternalOutput")
        posdbg = nc.dram_tensor("posdbg", [2, HD + 1, 512], mybir.dt.float32,
                                kind="ExternalOutput")
        rcdbg = nc.dram_tensor("rcdbg", [2, 512], mybir.dt.float32,
                               kind="ExternalOutput")
        rcbdbg = nc.dram_tensor("rcbdbg", [2, HD, 512], mybir.dt.float32,
                                kind="ExternalOutput")
        prdbg = nc.dram_tensor("prdbg", [4, 128, 2, 512], bf16,
                               kind="ExternalOutput")

    with tile.TileContext(nc) as tc:
        with (
            nc.allow_low_precision(reason="bf16 intermediate precision"),
            tc.tile_pool(name="singles", bufs=1) as singles,
            tc.tile_pool(name="qkv", bufs=1) as qkv,
            tc.tile_pool(name="xp", bufs=4) as xp,
            tc.tile_pool(name="v1p", bufs=1) as v1p,
            tc.tile_pool(name="pp", bufs=6) as pp,
            tc.tile_pool(name="np_", bufs=2) as np_,
            tc.tile_pool(name="fo", bufs=2) as fo,
            tc.tile_pool(name="psA", bufs=1, space="PSUM") as psA,
            tc.tile_pool(name="psS", bufs=2, space="PSUM") as psS,
            tc.tile_pool(name="psO", bufs=2, space="PSUM") as psO,
        ):
            w_sb = singles.tile([128, EC, 3 * SEC], bf16)
            wqr = wqkvT.rearrange("p (ec c) -> p ec c", ec=EC)
            nc.scalar.dma_start(out=w_sb[:], in_=wqr[:])
            woT_sb = singles.tile([128, D], bf16)
            nc.scalar.dma_start(out=woT_sb[:], in_=woT[:])
            ident128 = singles.tile([128, 128], bf16)
            nc.scalar.dma_start(out=ident128[:], in_=ident[:, 0:128])

            # vv[b][p, c, lh, 0:64] = V[b, 128c+p, 64lh:64lh+64];
            # [..., 64] = 1 (softmax denominator). Allocated and
            # ones-filled up front; PE transposes fill the V parts later.
            vvs = []
            for b in range(B):
                vv = v1p.tile([128, KC, 2, 128], bf16, tag=f"vv{b}",
                              name=f"vv{b}")
                nc.vector.memset(vv[:, :, :, HD:128], 0.0)
                nc.vector.memset(vv[:, :, :, HD], 1.0)
                vvs.append(vv)

            xfull = qkv.tile([128, EC, BS], bf16)
            xTr = xT.rearrange("(ec p) s -> p ec s", p=128)
            # ec-major per quad: matches quad-pass consumption order
            for half in range(2):
                ssl = slice(2048 * half, 2048 * (half + 1))
                for ec in range(EC):
                    nc.sync.dma_start(out=xfull[:, ec, ssl],
                                      in_=xTr[:, ec, ssl])
            qT = qkv.tile([128, BS], bf16)
            kT = qkv.tile([128, BS], bf16)
            vT = qkv.tile([128, BS], bf16)
            ocat_bt = [qkv.tile([128, 512], bf16, name=f"ocat{n}")
                       for n in range(NT)]

            # ---- stage A: qkvT projections ----
            # pair-groups, ec-outer: stationary w reused across both
            # n-tiles of a group; 2 PSUM banks (pa0/pa1)
            def stage_a(half, wi):
                dst = (qT, kT, vT)[wi]
                wsl = slice(128 * wi, 128 * (wi + 1))
                for pair in range(2):
                    n0 = 4 * half + 2 * pair
                    pts = [psA.tile([128, 512], f32, tag=f"pa{nn}",
                                    name=f"pa{half}{wi}{pair}{nn}")
                           for nn in range(2)]
                    for ec in range(EC):
                        for nn in range(2):
                            n = n0 + nn
                            nc.tensor.matmul(
                                pts[nn][:], w_sb[:, ec, wsl],
                                xfull[:, ec, 512 * n:512 * (n + 1)],
                                start=(ec == 0), stop=(ec == EC - 1))
                    for nn in range(2):
                        n = n0 + nn
                        nc.vector.tensor_copy(
                            dst[:, 512 * n:512 * (n + 1)], pts[nn][:])

            def make_v1(b):
                base = S * b
                vv = vvs[b]
                for i in range(KC):
                    pt = psA.tile([128, 128], bf16, tag=f"pa{i % 2}",
                                  name=f"pt{b}{i}")
                    nc.tensor.transpose(
                        pt[:],
                        vT[:, base + 128 * i:base + 128 * (i + 1)],
                        ident128[:])
                    ptv = pt.rearrange("p (lh h) -> p lh h", lh=2)
                    nc.vector.tensor_copy(vv[:, i, :, 0:HD], ptv)

            def attn_t(b, t):
                base = S * b
                njc = 4 * t + 4 if mask_mode == "causal" else KC
                pos = [psO.tile([128, 512], f32, tag="po",
                                name=f"po{b}{t}{_lh}") for _lh in range(2)]
                avq = []

                def do_av(j, qoff, pr):
                    for lh in range(2):
                        nc.tensor.matmul(pos[lh][:, qoff:512],
                                         vvs[b][:, j, lh, :],
                                         pr[:, lh, qoff:512],
                                         start=(j == 0), stop=(j == njc - 1))

                for j in range(njc):
                    qoff = max(0, 128 * (j - 4 * t)) \
                        if mask_mode == "causal" and not _NOQOFF else 0
                    ksl = slice(base + 128 * j, base + 128 * (j + 1))
                    qsl = slice(base + 512 * t + qoff, base + 512 * (t + 1))
                    ps = psS.tile([128, 2, 512], f32, tag="ps")
                    for lh in range(2):
                        hsl = slice(64 * lh, 64 * (lh + 1))
                        nc.tensor.matmul(ps[:, lh, qoff:512],
                                         kT[hsl, ksl], qT[hsl, qsl],
                                         start=True, stop=True)
                    pr = pp.tile([128, 2, 512], bf16, tag="pr")
                    nc.scalar.activation(pr[:, :, qoff:512],
                                         ps[:, :, qoff:512], Exp)
                    if mask_mode == "causal" and j >= 4 * t:
                        if _NOQOFF:
                            jm = j - 4 * t
                            nc.gpsimd.affine_select(
                                out=pr[:], in_=pr[:],
                                compare_op=mybir.AluOpType.is_ge,
                                fill=0.0, base=-128 * jm,
                                channel_multiplier=-1,
                                pattern=[[0, 2], [1, 512]])
                        else:
                            nc.gpsimd.affine_select(
                                out=pr[:, :, qoff:qoff + 128],
                                in_=pr[:, :, qoff:qoff + 128],
                                compare_op=mybir.AluOpType.is_ge,
                                fill=0.0, base=0,
                                channel_multiplier=-1,
                                pattern=[[0, 2], [1, 128]])
                    elif mask_mode == "general":
                        msk = xp.tile([128, 512], bf16, tag="msk")
                        nc.sync.dma_start(
                            out=msk[:],
                            in_=maskT[128 * j:128 * (j + 1),
                                      512 * t:512 * (t + 1)])
                        for lh in range(2):
                            sub = pr[:, lh, :]
                            nc.vector.tensor_mul(sub, sub, msk[:])
                    if _DEBUG and b == 0 and t == 0:
                        nc.gpsimd.dma_start(out=prdbg[j], in_=pr[:])
                    avq.append((j, qoff, pr))
                    if len(avq) > 1:
                        do_av(*avq.pop(0))
                while avq:
                    do_av(*avq.pop(0))
                # evict pos and compute reciprocals now (DVE); the
                # gpsimd broadcast+multiply is deferred one t-step so it
                # never blocks the next tile's affine_selects in the
                # gpsimd FIFO.
                if _DEBUG and b == 0 and t == 0:
                    for lh in range(2):
                        stg = np_.tile([HD + 1, 512], f32, tag="stg",
                                       name=f"stg{lh}")
                        nc.vector.tensor_copy(stg[:], pos[lh][:])
                        nc.gpsimd.dma_start(out=posdbg[lh], in_=stg[:])
                for lh in range(2):
                    ol = np_.tile([HD, 512], f32, tag="ol", bufs=4,
                                  name=f"ol{b}{t}{lh}")
                    nc.vector.tensor_copy(ol[:], pos[lh][0:HD, :])
                    lc = np_.tile([1, 512], f32, tag="lc")
                    nc.vector.tensor_copy(lc[:], pos[lh][HD:HD + 1, :])
                    rc = np_.tile([1, 512], f32, tag="rc", bufs=4,
                                  name=f"rc{b}{t}{lh}")
                    nc.vector.reciprocal_approx_fast(rc[:], lc[:])
                    njobs.append((b, t, lh, ol, rc))

            njobs = []

            def flush_norm(keep=0, final=False):
                while len(njobs) > keep:
                    b, t, lh, ol, rc = njobs.pop(0)
                    hsl = slice(64 * lh, 64 * (lh + 1))
                    rcb = np_.tile([HD, 512], f32, tag="rcb",
                                   name=f"rcb{b}{t}{lh}")
                    nc.gpsimd.partition_broadcast(rcb[:], rc[:])
                    # final muls go on gpsimd: no affines follow, and the
                    # DVE queue is several us deep at the kernel tail
                    eng = nc.gpsimd if final else nc.vector
                    eng.tensor_mul(ocat_bt[4 * b + t][hsl, :],
                                   ol[:], rcb[:])

            # out_projT partial for n-tile n = 4b + t, written bf16
            out_r = out_pT.rearrange("(oc p) s -> p oc s", p=128)

            def stage_c(b, t, split=False):
                n = 4 * b + t
                ssl = slice(512 * n, 512 * (n + 1))
                ft = fo.tile([128, EC, 512], bf16, tag="ft")
                for oc in range(EC):
                    osl = slice(128 * oc, 128 * (oc + 1))
                    pf = psA.tile([128, 512], f32, tag=f"pa{oc % 2}",
                                  name=f"pf{n}{oc}")
                    nc.tensor.matmul(pf[:], woT_sb[:, osl],
                                     ocat_bt[n][:],
                                     start=True, stop=True)
                    if split and oc % 2 == 0:
                        # ACT is idle at the kernel tail; halve the DVE
                        # eviction chain for the last jobs
                        nc.scalar.copy(ft[:, oc, :], pf[:])
                    else:
                        nc.vector.tensor_copy(ft[:, oc, :], pf[:])
                    if oc % 2 == 1:
                        nc.sync.dma_start(
                            out=out_r[:, oc - 1:oc + 1, ssl],
                            in_=ft[:, oc - 1:oc + 1, :])

            def dump_debug():
                if not _DEBUG:
                    return
                for wi, src_t in enumerate((qT, kT, vT)):
                    nc.gpsimd.dma_start(out=qkvdbg[wi], in_=src_t[:])
                for b in range(B):
                    vvf = vvs[b].rearrange("p c lh h -> p (c lh h)")
                    nc.gpsimd.dma_start(out=v1dbg[b], in_=vvf)
                for n in range(NT):
                    nc.gpsimd.dma_start(
                        out=ocatdbg[:, 512 * n:512 * (n + 1)],
                        in_=ocat_bt[n][:])

            # ---- emission schedule ----
            # half-0 qkv (b0) -> b0 attention interleaved with half-1
            # qkv passes (b1) so ACT exp starts early and qkv matmuls
            # absorb PE exp-wait bubbles; out-proj jobs lag attention
            # by 3 t-steps to stay off the PE critical path.
            for wi in range(3):
                stage_a(0, wi)
            make_v1(0)
            cjobs = []

            def push_c(b, t):
                flush_norm(keep=2)
                cjobs.append((b, t))
                if len(cjobs) >= 3:
                    stage_c(*cjobs.pop(0))

            attn_t(0, 0)
            push_c(0, 0)
            stage_a(1, 0)
            attn_t(0, 1)
            push_c(0, 1)
            stage_a(1, 1)
            attn_t(0, 2)
            push_c(0, 2)
            stage_a(1, 2)
            make_v1(1)
            attn_t(0, 3)
            push_c(0, 3)
            for t in range(QT):
                attn_t(1, t)
                push_c(1, t)
            flush_norm(final=True)
            while cjobs:
                stage_c(*cjobs.pop(0), split=True)
            dump_debug()

    nc.compile()
    return nc


def _classify_mask(mask):
    m = np.asarray(mask).reshape(S, S) != 0
    if m.all():
        return "none", None
    if np.array_equal(m, np.tril(np.ones((S, S), bool))):
        return "causal", None
    return "general", m.T.astype(np.float32)


def _ensure_ntff_hook():
    """Register antenv.axon_hooks with a ctypes NTFF profile hook if the
    container image lacks it (mirrors trn_agent_boot's registration)."""
    import types
    try:
        from antenv.axon_hooks import get_axon_ntff_profile_hook  # noqa: F401
        return
    except ImportError:
        pass
    import contextlib
    import ctypes

    hook = None
    so_path = "/opt/axon/libaxon_pjrt.so"
    try:
        lib = ctypes.CDLL(so_path)
        if hasattr(lib, "axon_start_nrt_profile"):
            lib.axon_start_nrt_profile.argtypes = [
                ctypes.POINTER(ctypes.c_int64), ctypes.c_size_t]
            lib.axon_start_nrt_profile.restype = ctypes.c_int64
            lib.axon_stop_nrt_profile.argtypes = [ctypes.c_char_p]
            lib.axon_stop_nrt_profile.restype = ctypes.c_int64

            @contextlib.contextmanager
            def _hook(output_dir, device_ids):
                import jax
                jax.devices()
                if device_ids:
                    ids = (ctypes.c_int64 * len(device_ids))(*device_ids)
                    rc = lib.axon_start_nrt_profile(ids, len(device_ids))
                else:
                    rc = lib.axon_start_nrt_profile(None, 0)
                if rc != 0:
                    raise RuntimeError(f"axon_start_nrt_profile rc={rc}")
                try:
                    yield
                finally:
                    n = lib.axon_stop_nrt_profile(str(output_dir).encode())
                    print(f"profile: {n} file(s) written to {output_dir}",
                          flush=True)

            hook = _hook
    except OSError:
        pass

    mod = types.ModuleType("antenv.axon_hooks")
    _h = [hook]
    mod.get_axon_ntff_profile_hook = lambda: _h[0]

    def _set(h):
        _h[0] = h

    mod.set_axon_ntff_profile_hook = _set
    sys.modules["antenv.axon_hooks"] = mod
    try:
        import antenv
        antenv.axon_hooks = mod
    except ImportError:
        pass


def _core_inputs(query, mask_mode, maskT, W_qkv, W_out):
    x = np.ascontiguousarray(
        np.asarray(query, np.float32).reshape(BS, D))
    xT_bf = np.ascontiguousarray(x.T).astype(BF16)
    W_qkv = np.asarray(W_qkv, np.float32)
    W_out = np.asarray(W_out, np.float32)

    in_maps = []
    for c in range(NCORES):
        sl = slice(SEC * c, SEC * (c + 1))
        wq = W_qkv[sl, :].T * np.float32(HD ** -0.5)
        wk = W_qkv[D + SEC * c:D + SEC * (c + 1), :].T
        wv = W_qkv[2 * D + SEC * c:2 * D + SEC * (c + 1), :].T
        ident = np.zeros((128, 128 + 2 * KC), BF16)
        ident[:, 0:128] = np.eye(128, dtype=BF16)
        ident[:, 128:128 + 2 * KC] = 1.0
        wcat = np.concatenate([wq, wk, wv], axis=1,
                              dtype=np.float32)  # [D, 3*SEC]
        # partition-major: wp[p, ec, c] = wcat[128*ec + p, c]
        wp = wcat.reshape(EC, 128, 3 * SEC).transpose(1, 0, 2)
        m = {
            "xT": xT_bf,
            "ident": ident,
            "wqkvT": np.ascontiguousarray(
                wp.reshape(128, EC * 3 * SEC)).astype(BF16),
            "woT": np.ascontiguousarray(W_out[:, sl].T).astype(BF16),
        }
        if mask_mode == "general":
            m["maskT"] = maskT.astype(BF16)
        in_maps.append(m)
    return in_maps


def kernel(key, query, value, mask, W_qkv, W_out, b_out):
    from concourse.bass_utils import run_bass_kernel_spmd
    import os

    mask_mode, maskT = _classify_mask(mask)
    if mask_mode not in _cache:
        _cache[mask_mode] = _build(mask_mode)
    nc = _cache[mask_mode]

    in_maps = _core_inputs(query, mask_mode, maskT, W_qkv, W_out)

    trace = bool(int(os.environ.get("KERNEL_TRACE", "0")))
    if trace:
        _ensure_ntff_hook()
        try:
            res = run_bass_kernel_spmd(nc, in_maps,
                                       core_ids=list(range(NCORES)),
                                       trace=True)
        except Exception as e:
            print(f"traced run failed ({e!r}); retrying untraced",
                  flush=True)
            res = run_bass_kernel_spmd(nc, in_maps,
                                       core_ids=list(range(NCORES)))
        print(f"HW exec time: {res.exec_time_ns} ns", flush=True)
        kernel.last_exec_ns = res.exec_time_ns
        kernel.last_results = res
    else:
        res = run_bass_kernel_spmd(nc, in_maps, core_ids=list(range(NCORES)))

    acc = res.results[0]["out_pT"].astype(np.float32)
    for c in range(1, NCORES):
        acc = acc + res.results[c]["out_pT"].astype(np.float32)
    out = acc.T.reshape(B, S, D) + np.asarray(b_out, np.float32)
    return out.astype(np.float32)


# revision 29
# speedup vs baseline: 1.1197x; 1.1197x over previous
"""Multi-head attention forward, sharded head-parallel across 8 NeuronCores.

Per core c (heads 2c, 2c+1):
  qT/kT/vT = (x @ W{q,k,v}_c.T).T   quad-pass matmuls (ec-outer) for
                                    stationary reuse, ec-major DMA
  v1       = V in [kpos, hd] layout via DMA-engine transpose (off PE)
  scoresT  = kT_chunk.T @ qT        exact-causal: only cols [qoff,512)
  probsT   = exp(scoresT)           ACT on the live columns only
  triangle mask                     gpsimd affine_select on 128-col diag
  av+rowsum: out.T = [v | 1].T @ probsT  (ones column -> softmax denom)
  normalize: DVE reciprocal + gpsimd partition_broadcast + DVE mul
  out_projT partial per (b,t), interleaved with attention, bf16 output
Host: sum the 8 partial [1024, 4096] outputs, transpose, add bias.
"""
import sys

sys.path.insert(0, "/opt/trn_rl_repo")

import ml_dtypes
import numpy as np

BF16 = ml_dtypes.bfloat16

B, S, D = 2, 2048, 1024
H, HD = 16, 64
NCORES = 8
SEC = 128           # output dims per core per section (2 heads * 64)
BS = B * S          # 4096
NT = BS // 512      # 8 seq tiles of 512
EC = D // 128       # 8 embed chunks
QT = S // 512       # 4 q-tiles per (b,h)
KC = S // 128       # 16 k-chunks per (b,h)

_cache = {}

import os as _os
_NOQOFF = bool(int(_os.environ.get("KV_NOQOFF", "0")))
_DEBUG = bool(int(_os.environ.get("KV_DEBUG", "0")))


def _build(mask_mode):
    import concourse.bass as bass
    import concourse.tile as tile
    from concourse import bacc, mybir

    f32 = mybir.dt.float32
    bf16 = mybir.dt.bfloat16
    Exp = mybir.ActivationFunctionType.Exp

    nc = bacc.Bacc("TRN2", target_bir_lowering=False, debug=False,
                   num_devices=NCORES)

    xT = nc.dram_tensor("xT", [D, BS], bf16, kind="ExternalInput")
    # host pre-packs to partition-major [128, EC, 3*SEC] for a
    # contiguous-per-partition (descriptor-efficient) load
    wqkvT = nc.dram_tensor("wqkvT", [128, EC * 3 * SEC], bf16,
                           kind="ExternalInput")
    woT = nc.dram_tensor("woT", [SEC, D], bf16, kind="ExternalInput")
    # [:, 0:128] identity (PE-mode V transposes); [:, 128:128+2*KC]
    # ones (softmax-denominator columns of vv)
    ident = nc.dram_tensor("ident", [128, 128 + 2 * KC], bf16,
                           kind="ExternalInput")
    if mask_mode == "general":
        maskT = nc.dram_tensor("maskT", [S, S], bf16, kind="ExternalInput")
    out_pT = nc.dram_tensor("out_pT", [D, BS], bf16, kind="ExternalOutput")
    if _DEBUG:
        qkvdbg = nc.dram_tensor("qkvdbg", [3, 128, BS], bf16,
                                kind="ExternalOutput")
        v1dbg = nc.dram_tensor("v1dbg", [B, 128, KC * 2 * 128], bf16,
                               kind="ExternalOutput")
        ocatdbg = nc.dram_tensor("ocatdbg", [128, BS], bf16,
                                 kind="ExternalOutput")
        posdbg = nc.dram_tensor("posdbg", [2, HD + 1, 512], mybir.dt.float32,
                                kind="ExternalOutput")
        rcdbg = nc.dram_tensor("rcdbg", [2, 512], mybir.dt.float32,
                               kind="ExternalOutput")
        rcbdbg = nc.dram_tensor("rcbdbg", [2, HD, 512], mybir.dt.float32,
                                kind="ExternalOutput")
        prdbg = nc.dram_tensor("prdbg", [4, 128, 2, 512], bf16,
                               kind="ExternalOutput")

    with tile.TileContext(nc) as tc:
        with (
            nc.allow_low_precision(reason="bf16 intermediate precision"),
            tc.tile_pool(name="singles", bufs=1) as singles,
            tc.tile_pool(name="qkv", bufs=1) as qkv,
            tc.tile_pool(name="xp", bufs=4) as xp,
            tc.tile_pool(name="v1p", bufs=1) as v1p,
            tc.tile_pool(name="pp", bufs=6) as pp,
            tc.tile_pool(name="np_", bufs=2) as np_,
            tc.tile_pool(name="fo", bufs=2) as fo,
            tc.tile_pool(name="psA", bufs=1, space="PSUM") as psA,
            tc.tile_pool(name="psS", bufs=2, space="PSUM") as psS,
            tc.tile_pool(name="psO", bufs=2, space="PSUM") as psO,
        ):
            w_sb = singles.tile([128, EC, 3 * SEC], bf16)
            wqr = wqkvT.rearrange("p (ec c) -> p ec c", ec=EC)
            nc.scalar.dma_start(out=w_sb[:], in_=wqr[:])
            woT_sb = singles.tile([128, D], bf16)
            nc.scalar.dma_start(out=woT_sb[:], in_=woT[:])
            ident128 = singles.tile([128, 128], bf16)
            nc.scalar.dma_start(out=ident128[:], in_=ident[:, 0:128])

            # vv[b][p, c, lh, 0:64] = V[b, 128c+p, 64lh:64lh+64];
            # [..., 64] = 1 (softmax denominator). Allocated and
            # ones-filled up front; PE transposes fill the V parts later.
            vvs = []
            for b in range(B):
                vv = v1p.tile([128, KC, 2, 128], bf16, tag=f"vv{b}",
                              name=f"vv{b}")
                nc.vector.memset(vv[:, :, :, HD:128], 0.0)
                nc.vector.memset(vv[:, :, :, HD], 1.0)
                vvs.append(vv)

            xfull = qkv.tile([128, EC, BS], bf16)
            xTr = xT.rearrange("(ec p) s -> p ec s", p=128)
            # ec-major per quad: matches quad-pass consumption order
            for half in range(2):
                ssl = slice(2048 * half, 2048 * (half + 1))
                for ec in range(EC):
                    nc.sync.dma_start(out=xfull[:, ec, ssl],
                                      in_=xTr[:, ec, ssl])
            qT = qkv.tile([128, BS], bf16)
            kT = qkv.tile([128, BS], bf16)
            vT = qkv.tile([128, BS], bf16)
            ocat_bt = [qkv.tile([128, 512], bf16, name=f"ocat{n}")
                       for n in range(NT)]

            # ---- stage A: qkvT projections ----
            # pair-groups, ec-outer: stationary w reused across both
            # n-tiles of a group; 2 PSUM banks (pa0/pa1)
            def stage_a(half, wi):
                dst = (qT, kT, vT)[wi]
                wsl = slice(128 * wi, 128 * (wi + 1))
                for pair in range(2):
                    n0 = 4 * half + 2 * pair
                    pts = [psA.tile([128, 512], f32, tag=f"pa{nn}",
                                    name=f"pa{half}{wi}{pair}{nn}")
                           for nn in range(2)]
                    for ec in range(EC):
                        for nn in range(2):
                            n = n0 + nn
                            nc.tensor.matmul(
                                pts[nn][:], w_sb[:, ec, wsl],
                                xfull[:, ec, 512 * n:512 * (n + 1)],
                                start=(ec == 0), stop=(ec == EC - 1))
                    for nn in range(2):
                        n = n0 + nn
                        nc.vector.tensor_copy(
                            dst[:, 512 * n:512 * (n + 1)], pts[nn][:])

            def make_v1(b):
                base = S * b
                vv = vvs[b]
                for i in range(KC):
                    pt = psA.tile([128, 128], bf16, tag=f"pa{i % 2}",
                                  name=f"pt{b}{i}")
                    nc.tensor.transpose(
                        pt[:],
                        vT[:, base + 128 * i:base + 128 * (i + 1)],
                        ident128[:])
                    ptv = pt.rearrange("p (lh h) -> p lh h", lh=2)
                    nc.vector.tensor_copy(vv[:, i, :, 0:HD], ptv)

            def attn_t(b, t):
                base = S * b
                njc = 4 * t + 4 if mask_mode == "causal" else KC
                pos = [psO.tile([128, 512], f32, tag="po",
                                name=f"po{b}{t}{_lh}") for _lh in range(2)]
                avq = []

                def do_av(j, qoff, pr):
                    for lh in range(2):
                        nc.tensor.matmul(pos[lh][:, qoff:512],
                                         vvs[b][:, j, lh, :],
                                         pr[:, lh, qoff:512],
                                         start=(j == 0), stop=(j == njc - 1))

                for j in range(njc):
                    qoff = max(0, 128 * (j - 4 * t)) \
                        if mask_mode == "causal" and not _NOQOFF else 0
                    ksl = slice(base + 128 * j, base + 128 * (j + 1))
                    qsl = slice(base + 512 * t + qoff, base + 512 * (t + 1))
                    ps = psS.tile([128, 2, 512], f32, tag="ps")
                    for lh in range(2):
                        hsl = slice(64 * lh, 64 * (lh + 1))
                        nc.tensor.matmul(ps[:, lh, qoff:512],
                                         kT[hsl, ksl], qT[hsl, qsl],
                                         start=True, stop=True)
                    pr = pp.tile([128, 2, 512], bf16, tag="pr")
                    nc.scalar.activation(pr[:, :, qoff:512],
                                         ps[:, :, qoff:512], Exp)
                    if mask_mode == "causal" and j >= 4 * t:
                        if _NOQOFF:
                            jm = j - 4 * t
                            nc.gpsimd.affine_select(
                                out=pr[:], in_=pr[:],
                                compare_op=mybir.AluOpType.is_ge,
                                fill=0.0, base=-128 * jm,
                                channel_multiplier=-1,
                                pattern=[[0, 2], [1, 512]])
                        else:
                            nc.gpsimd.affine_select(
                                out=pr[:, :, qoff:qoff + 128],
                                in_=pr[:, :, qoff:qoff + 128],
                                compare_op=mybir.AluOpType.is_ge,
                                fill=0.0, base=0,
                                channel_multiplier=-1,
                                pattern=[[0, 2], [1, 128]])
                    elif mask_mode == "general":
                        msk = xp.tile([128, 512], bf16, tag="msk")
                        nc.sync.dma_start(
                            out=msk[:],
                            in_=maskT[128 * j:128 * (j + 1),
                                      512 * t:512 * (t + 1)])
                        for lh in range(2):
                            sub = pr[:, lh, :]
                            nc.vector.tensor_mul(sub, sub, msk[:])
                    if _DEBUG and b == 0 and t == 0:
                        nc.gpsimd.dma_start(out=prdbg[j], in_=pr[:])
                    avq.append((j, qoff, pr))
                    if len(avq) > 1:
                        do_av(*avq.pop(0))
                while avq:
                    do_av(*avq.pop(0))
                # evict pos and compute reciprocals now (DVE); the
                # gpsimd broadcast+multiply is deferred one t-step so it
                # never blocks the next tile's affine_selects in the
                # gpsimd FIFO.
                if _DEBUG and b == 0 and t == 0:
                    for lh in range(2):
                        stg = np_.tile([HD + 1, 512], f32, tag="stg",
                                       name=f"stg{lh}")
                        nc.vector.tensor_copy(stg[:], pos[lh][:])
                        nc.gpsimd.dma_start(out=posdbg[lh], in_=stg[:])
                for lh in range(2):
                    ol = np_.tile([HD, 512], f32, tag="ol", bufs=4,
                                  name=f"ol{b}{t}{lh}")
                    nc.vector.tensor_copy(ol[:], pos[lh][0:HD, :])
                    lc = np_.tile([1, 512], f32, tag="lc")
                    nc.vector.tensor_copy(lc[:], pos[lh][HD:HD + 1, :])
                    rc = np_.tile([1, 512], f32, tag="rc", bufs=4,
                                  name=f"rc{b}{t}{lh}")
                    nc.vector.reciprocal_approx_fast(rc[:], lc[:])
                    njobs.append((b, t, lh, ol, rc))

            njobs = []

            def flush_norm(keep=0):
                while len(njobs) > keep:
                    b, t, lh, ol, rc = njobs.pop(0)
                    hsl = slice(64 * lh, 64 * (lh + 1))
                    rcb = np_.tile([HD, 512], f32, tag="rcb",
                                   name=f"rcb{b}{t}{lh}")
                    nc.gpsimd.partition_broadcast(rcb[:], rc[:])
                    nc.vector.tensor_mul(ocat_bt[4 * b + t][hsl, :],
                                         ol[:], rcb[:])

            # out_projT partial for n-tile n = 4b + t, written bf16
            out_r = out_pT.rearrange("(oc p) s -> p oc s", p=128)

            def stage_c(b, t):
                n = 4 * b + t
                ssl = slice(512 * n, 512 * (n + 1))
                ft = fo.tile([128, EC, 512], bf16, tag="ft")
                for oc in range(EC):
                    osl = slice(128 * oc, 128 * (oc + 1))
                    pf = psA.tile([128, 512], f32, tag=f"pa{oc % 2}",
                                  name=f"pf{n}{oc}")
                    nc.tensor.matmul(pf[:], woT_sb[:, osl],
                                     ocat_bt[n][:],
                                     start=True, stop=True)
                    nc.vector.tensor_copy(ft[:, oc, :], pf[:])
                    if oc % 2 == 1:
                        nc.sync.dma_start(
                            out=out_r[:, oc - 1:oc + 1, ssl],
                            in_=ft[:, oc - 1:oc + 1, :])

            def dump_debug():
                if not _DEBUG:
                    return
                for wi, src_t in enumerate((qT, kT, vT)):
                    nc.gpsimd.dma_start(out=qkvdbg[wi], in_=src_t[:])
                for b in range(B):
                    vvf = vvs[b].rearrange("p c lh h -> p (c lh h)")
                    nc.gpsimd.dma_start(out=v1dbg[b], in_=vvf)
                for n in range(NT):
                    nc.gpsimd.dma_start(
                        out=ocatdbg[:, 512 * n:512 * (n + 1)],
                        in_=ocat_bt[n][:])

            # ---- emission schedule ----
            # half-0 qkv (b0) -> b0 attention interleaved with half-1
            # qkv passes (b1) so ACT exp starts early and qkv matmuls
            # absorb PE exp-wait bubbles; out-proj jobs lag attention
            # by 3 t-steps to stay off the PE critical path.
            for wi in range(3):
                stage_a(0, wi)
            make_v1(0)
            cjobs = []

            def push_c(b, t):
                flush_norm(keep=2)
                cjobs.append((b, t))
                if len(cjobs) >= 3:
                    stage_c(*cjobs.pop(0))

            attn_t(0, 0)
            push_c(0, 0)
            stage_a(1, 0)
            attn_t(0, 1)
            push_c(0, 1)
            stage_a(1, 1)
            attn_t(0, 2)
            push_c(0, 2)
            stage_a(1, 2)
            make_v1(1)
            attn_t(0, 3)
            push_c(0, 3)
            for t in range(QT):
                attn_t(1, t)
                push_c(1, t)
            flush_norm(keep=2)
            if len(cjobs) > 2:
                stage_c(*cjobs.pop(0))
            flush_norm()
            while cjobs:
                stage_c(*cjobs.pop(0))
            dump_debug()

    nc.compile()
    return nc


def _classify_mask(mask):
    m = np.asarray(mask).reshape(S, S) != 0
    if m.all():
        return "none", None
    if np.array_equal(m, np.tril(np.ones((S, S), bool))):
        return "causal", None
    return "general", m.T.astype(np.float32)


def _ensure_ntff_hook():
    """Register antenv.axon_hooks with a ctypes NTFF profile hook if the
    container image lacks it (mirrors trn_agent_boot's registration)."""
    import types
    try:
        from antenv.axon_hooks import get_axon_ntff_profile_hook  # noqa: F401
        return
    except ImportError:
        pass
    import contextlib
    import ctypes

    hook = None
    so_path = "/opt/axon/libaxon_pjrt.so"
    try:
        lib = ctypes.CDLL(so_path)
        if hasattr(lib, "axon_start_nrt_profile"):
            lib.axon_start_nrt_profile.argtypes = [
                ctypes.POINTER(ctypes.c_int64), ctypes.c_size_t]
            lib.axon_start_nrt_profile.restype = ctypes.c_int64
            lib.axon_stop_nrt_profile.argtypes = [ctypes.c_char_p]
            lib.axon_stop_nrt_profile.restype = ctypes.c_int64

            @contextlib.contextmanager
            def _hook(output_dir, device_ids):
                import jax
                jax.devices()
                if device_ids:
                    ids = (ctypes.c_int64 * len(device_ids))(*device_ids)
                    rc = lib.axon_start_nrt_profile(ids, len(device_ids))
                else:
                    rc = lib.axon_start_nrt_profile(None, 0)
                if rc != 0:
                    raise RuntimeError(f"axon_start_nrt_profile rc={rc}")
                try:
                    yield
                finally:
                    n = lib.axon_stop_nrt_profile(str(output_dir).encode())
                    print(f"profile: {n} file(s) written to {output_dir}",
                          flush=True)

            hook = _hook
    except OSError:
        pass

    mod = types.ModuleType("antenv.axon_hooks")
    _h = [hook]
    mod.get_axon_ntff_profile_hook = lambda: _h[0]

    def _set(h):
        _h[0] = h

    mod.set_axon_ntff_profile_hook = _set
    sys.modules["antenv.axon_hooks"] = mod
    try:
        import antenv
        antenv.axon_hooks = mod
    except ImportError:
        pass


def _core_inputs(query, mask_mode, maskT, W_qkv, W_out):
    x = np.ascontiguousarray(
        np.asarray(query, np.float32).reshape(BS, D))
    xT_bf = np.ascontiguousarray(x.T).astype(BF16)
    W_qkv = np.asarray(W_qkv, np.float32)
    W_out = np.asarray(W_out, np.float32)

    in_maps = []
    for c in range(NCORES):
        sl = slice(SEC * c, SEC * (c + 1))
        wq = W_qkv[sl, :].T * np.float32(HD ** -0.5)
        wk = W_qkv[D + SEC * c:D + SEC * (c + 1), :].T
        wv = W_qkv[2 * D + SEC * c:2 * D + SEC * (c + 1), :].T
        ident = np.zeros((128, 128 + 2 * KC), BF16)
        ident[:, 0:128] = np.eye(128, dtype=BF16)
        ident[:, 128:128 + 2 * KC] = 1.0
        wcat = np.concatenate([wq, wk, wv], axis=1,
                              dtype=np.float32)  # [D, 3*SEC]
        # partition-major: wp[p, ec, c] = wcat[128*ec + p, c]
        wp = wcat.reshape(EC, 128, 3 * SEC).transpose(1, 0, 2)
        m = {
            "xT": xT_bf,
            "ident": ident,
            "wqkvT": np.ascontiguousarray(
                wp.reshape(128, EC * 3 * SEC)).astype(BF16),
            "woT": np.ascontiguousarray(W_out[:, sl].T).astype(BF16),
        }
        if mask_mode == "general":
            m["maskT"] = maskT.astype(BF16)
        in_maps.append(m)
    return in_maps


def kernel(key, query, value, mask, W_qkv, W_out, b_out):
    from concourse.bass_utils import run_bass_kernel_spmd
    import os

    mask_mode, maskT = _classify_mask(mask)
    if mask_mode not in _cache:
        _cache[mask_mode] = _build(mask_mode)
    nc = _cache[mask_mode]

    in_maps = _core_inputs(query, mask_mode, maskT, W_qkv, W_out)

    trace = bool(int(os.environ.get("KERNEL_TRACE", "0")))
    if trace:
        _ensure_ntff_hook()
        try:
            res = run_bass_kernel_spmd(nc, in_maps,
                                       core_ids=list(range(NCORES)),
                                       trace=True)
        except Exception as e:
            print(f"traced run failed ({e!r}); retrying untraced",
                  flush=True)
            res = run_bass_kernel_spmd(nc, in_maps,
                                       core_ids=list(range(NCORES)))
        print(f"HW exec time: {res.exec_time_ns} ns", flush=True)
        kernel.last_exec_ns = res.exec_time_ns
        kernel.last_results = res
    else:
        res = run_bass_kernel_spmd(nc, in_maps, core_ids=list(range(NCORES)))

    acc = res.results[0]["out_pT"].astype(np.float32)
    for c in range(1, NCORES):
        acc = acc + res.results[c]["out_pT"].astype(np.float32)
    out = acc.T.reshape(B, S, D) + np.asarray(b_out, np.float32)
    return out.astype(np.float32)
